# revision 8
# baseline (speedup 1.0000x reference)
"""Trainium2 Bass kernel for nn_CosineLayer.

Computes out[b, o] = <x_b, w_o> / (max(||x_b||,eps) * max(||w_o||,eps))
for x [8192, 4096], weights [8192, 4096] -> out [8192, 8192], fp32.

Strategy (8 NeuronCores, tensor parallel):
  - Shard weights row-wise: core c gets w[c*1024:(c+1)*1024, :]; x replicated.
  - Host pre-transposes x and w to [K, .] layout so the contraction dim (K)
    lands on SBUF partitions with contiguous DMA (no fp32 PE transposes).
  - Per core: wT shard (16MB) resident in SBUF; xT streamed per 128-row
    m-tile; PSUM accumulation over 32 k-subtiles; matmuls in float32r
    (full-rate 4-byte matmul mode) or float32 (exact, 4 cycles/row).
  - Row norms: ACT square + DVE free-dim reduce + one PE matmul against a
    ones vector (puts ||x_b||^2 at psum[b, 0], the orientation the
    eviction needs). Norms are computed entirely in fp32.
  - Eviction: single fused DVE op out = (psum * invx[b]) * invw_bcast[:, o].
  - Host concatenates the 8 [8192, 1024] shards along axis 1.
"""

import sys

import numpy as np

for _p in ("/opt/trn_rl_repo", "/opt/pypackages"):
    if _p not in sys.path:
        sys.path.append(_p)

import concourse.bass as bass  # noqa: E402
import concourse.tile as tile  # noqa: E402
from concourse import bacc, mybir  # noqa: E402
from concourse.bass_utils import run_bass_kernel_spmd  # noqa: E402

P = 128
NCHUNK = 512  # psum bank free size (fp32)
EPS = 1e-8
N_CORES = 8

F32 = mybir.dt.float32
AF = mybir.ActivationFunctionType
ALU = mybir.AluOpType
AX = mybir.AxisListType


def build_cosine_bass(B, K, O, mm_f32r=True, kh_split=2):
    """Build the per-core Bass program.

    Per-core I/O:
      xT  [K, B]  fp32 (x transposed, replicated)
      wT  [K, O]  fp32 (this core's weight shard, transposed)
      out [B, O]  fp32
    """
    assert B % P == 0 and K % P == 0 and O % NCHUNK == 0
    KT = K // P        # k subtiles of 128
    MT = B // P        # output m-tiles of 128 rows
    OC = O // NCHUNK   # psum chunks per m-tile
    assert KT % kh_split == 0
    KH = KT // kh_split  # k subtiles per half-load

    mmdt = mybir.dt.float32r if mm_f32r else F32

    nc = bacc.Bacc("TRN2", target_bir_lowering=False, debug=False)
    xT = nc.dram_tensor("xT", [K, B], mmdt, kind="ExternalInput").ap()
    wT = nc.dram_tensor("wT", [K, O], mmdt, kind="ExternalInput").ap()
    out = nc.dram_tensor("out", [B, O], F32, kind="ExternalOutput").ap()

    with tile.TileContext(nc) as tc:
        with (
            tc.tile_pool(name="wpool", bufs=1) as wpool,
            tc.tile_pool(name="misc", bufs=1) as misc,
            tc.tile_pool(name="wsqp", bufs=2) as wsqp,
            tc.tile_pool(name="xpool", bufs=2) as xpool,
            tc.tile_pool(name="sqpool", bufs=1) as sqpool,
            tc.tile_pool(name="accp", bufs=2) as accp,
            tc.tile_pool(name="normp", bufs=2) as normp,
            tc.tile_pool(name="opool", bufs=3) as opool,
            tc.tile_pool(name="psum", bufs=2, space="PSUM") as psump,
            tc.tile_pool(name="psum_n", bufs=2, space="PSUM") as psump_n,
        ):
            ones = misc.tile([P, 1], F32)
            nc.vector.memset(ones[:], 1.0)

            # ---- resident weight shard, loaded per k-subtile ----
            wT_sb = wpool.tile([P, KT, O], mmdt)
            for kt in range(KT):
                nc.sync.dma_start(
                    wT_sb[:, kt, :],
                    wT[kt * P : (kt + 1) * P, :],
                )

            # ---- w norms: acc_w[p, o] = sum_kt wT[kt*P+p, o]^2 ----
            acc_w = misc.tile([P, O], F32)
            for kt in range(KT):
                wsq = wsqp.tile([P, O], F32, tag="wsq")
                nc.scalar.activation(wsq[:], wT_sb[:, kt, :].bitcast(F32), AF.Square)
                if kt == 0:
                    nc.vector.tensor_copy(acc_w[:], wsq[:])
                else:
                    nc.vector.tensor_add(acc_w[:], acc_w[:], wsq[:])

            # partition-reduce via ones-matmul, per 512-chunk (psum bank limit)
            invw_row = misc.tile([1, O], F32)
            for oc in range(OC):
                ps_w = psump_n.tile([1, NCHUNK], F32, tag="ps_w")
                nc.tensor.matmul(
                    ps_w[:],
                    lhsT=ones[:],
                    rhs=acc_w[:, oc * NCHUNK : (oc + 1) * NCHUNK],
                    start=True,
                    stop=True,
                )
                wn = misc.tile([1, NCHUNK], F32, tag="wn")
                nc.scalar.activation(wn[:], ps_w[:], AF.Sqrt)
                nc.vector.tensor_scalar_max(wn[:], wn[:], EPS)
                nc.vector.reciprocal(
                    invw_row[:, oc * NCHUNK : (oc + 1) * NCHUNK], wn[:]
                )
            # broadcast to all partitions for the eviction multiply
            invw_b = misc.tile([P, O], F32)
            nc.gpsimd.partition_broadcast(invw_b[:], invw_row[:])

            # ---- main loop over output m-tiles ----
            for m in range(MT):
                msl = slice(m * P, (m + 1) * P)

                xts = []
                for kh in range(kh_split):
                    xt = xpool.tile([P, KH, P], mmdt, tag="xt")
                    nc.sync.dma_start(
                        xt[:],
                        xT[kh * KH * P : (kh + 1) * KH * P, msl].rearrange(
                            "(kt p) b -> p kt b", p=P
                        ),
                    )
                    xts.append(xt)

                psums = [
                    psump.tile([P, NCHUNK], F32, tag=f"ps{oc}", name=f"ps{oc}")
                    for oc in range(OC)
                ]

                acchs = []
                for kh in range(kh_split):
                    xt = xts[kh]
                    for kt in range(KH):
                        for oc in range(OC):
                            nc.tensor.matmul(
                                psums[oc][:],
                                lhsT=xt[:, kt, :],
                                rhs=wT_sb[
                                    :,
                                    kh * KH + kt,
                                    oc * NCHUNK : (oc + 1) * NCHUNK,
                                ],
                                start=(kh == 0 and kt == 0),
                                stop=(kh == kh_split - 1 and kt == KH - 1),
                            )
                    # x-norm partials for this half
                    sq = sqpool.tile([P, KH, P], F32, tag="sq")
                    nc.scalar.activation(sq[:], xt[:].bitcast(F32), AF.Square)
                    acch = accp.tile([P, P], F32, tag=f"acch{kh}")
                    nc.vector.tensor_reduce(
                        acch[:], sq.rearrange("p kt b -> p b kt"), AX.X, ALU.add
                    )
                    acchs.append(acch)

                acc = acchs[0]
                for kh in range(1, kh_split):
                    acc2 = accp.tile([P, P], F32, tag="accsum")
                    nc.vector.tensor_add(acc2[:], acc[:], acchs[kh][:])
                    acc = acc2

                # sum over the 128 partitions: psum_n[b, 0] = ||x_b||^2
                ps_n = psump_n.tile([P, 1], F32, tag="ps_n")
                nc.tensor.matmul(ps_n[:], lhsT=acc[:], rhs=ones[:], start=True, stop=True)
                xn = normp.tile([P, 1], F32, tag="xn")
                nc.scalar.activation(xn[:], ps_n[:], AF.Sqrt)
                nc.vector.tensor_scalar_max(xn[:], xn[:], EPS)
                invx = normp.tile([P, 1], F32, tag="invx")
                nc.vector.reciprocal(invx[:], xn[:])

                for oc in range(OC):
                    ot = opool.tile([P, NCHUNK], F32, tag="ot")
                    nc.vector.scalar_tensor_tensor(
                        out=ot[:],
                        in0=psums[oc][:],
                        scalar=invx[:],
                        in1=invw_b[:, oc * NCHUNK : (oc + 1) * NCHUNK],
                        op0=ALU.mult,
                        op1=ALU.mult,
                    )
                    nc.sync.dma_start(out[msl, oc * NCHUNK : (oc + 1) * NCHUNK], ot[:])

    nc.compile()
    return nc


_NC_CACHE = {}


def _get_nc(B, K, O, mm_f32r=True):
    key = (B, K, O, mm_f32r)
    if key not in _NC_CACHE:
        _NC_CACHE[key] = build_cosine_bass(B, K, O, mm_f32r=mm_f32r)
    return _NC_CACHE[key]


def _run(x, weights, mm_f32r=True, n_cores=N_CORES, trace=False, **kw):
    B, K = x.shape
    Ofull = weights.shape[0]
    assert weights.shape[1] == K
    Oshard = Ofull // n_cores

    xT = np.ascontiguousarray(x.T).astype(np.float32, copy=False)
    wT = np.ascontiguousarray(weights.T).astype(np.float32, copy=False)

    nc = _get_nc(B, K, Oshard, mm_f32r=mm_f32r)
    in_maps = [
        {
            "xT": xT,
            "wT": np.ascontiguousarray(wT[:, c * Oshard : (c + 1) * Oshard]),
        }
        for c in range(n_cores)
    ]
    res = run_bass_kernel_spmd(nc, in_maps, list(range(n_cores)), trace=trace, **kw)
    out = np.concatenate([res.results[c]["out"] for c in range(n_cores)], axis=1)
    return out, res


def kernel(x, weights):
    out, _ = _run(np.asarray(x), np.asarray(weights))
    return out


# revision 19
# speedup vs baseline: 1.0256x; 1.0256x over previous
"""Trainium2 Bass kernel for nn_CosineLayer.

Computes out[b, o] = <x_b, w_o> / (max(||x_b||,eps) * max(||w_o||,eps))
for x [8192, 4096], weights [8192, 4096] -> out [8192, 8192], fp32.

Strategy (8 NeuronCores, tensor parallel):
  - Shard weights row-wise: core c gets w[c*1024:(c+1)*1024, :]; x replicated.
  - Host pre-transposes x and w to [K, .] layout so the contraction dim (K)
    lands on SBUF partitions with contiguous DMA (no fp32 PE transposes).
  - Per core: wT shard (16MB) resident in SBUF; xT streamed per 128-row
    m-tile; PSUM accumulation over 32 k-subtiles; matmuls in float32r
    (full-rate 4-byte matmul mode) or float32 (exact, 4 cycles/row).
  - Row norms: ACT square + DVE free-dim reduce + one PE matmul against a
    ones vector (puts ||x_b||^2 at psum[b, 0], the orientation the
    eviction needs). Norms are computed entirely in fp32.
  - Eviction: single fused DVE op out = (psum * invx[b]) * invw_bcast[:, o].
  - Host concatenates the 8 [8192, 1024] shards along axis 1.
"""

import sys

import numpy as np

for _p in ("/opt/trn_rl_repo", "/opt/pypackages"):
    if _p not in sys.path:
        sys.path.append(_p)

import concourse.bass as bass  # noqa: E402
import concourse.tile as tile  # noqa: E402
from concourse import bacc, mybir  # noqa: E402
from concourse.bass_utils import run_bass_kernel_spmd  # noqa: E402

P = 128
NCHUNK = 512  # psum bank free size (fp32)
EPS = 1e-8
N_CORES = 8

F32 = mybir.dt.float32
AF = mybir.ActivationFunctionType
ALU = mybir.AluOpType
AX = mybir.AxisListType


def build_cosine_bass(B, K, O, mm_f32r=True, kh_split=4):
    """Build the per-core Bass program.

    Per-core I/O:
      xT  [K, B]  fp32 (x transposed, replicated)
      wT  [K, O]  fp32 (this core's weight shard, transposed)
      out [B, O]  fp32
    """
    assert B % P == 0 and K % P == 0 and O % NCHUNK == 0
    KT = K // P        # k subtiles of 128
    MT = B // P        # output m-tiles of 128 rows
    OC = O // NCHUNK   # psum chunks per m-tile
    assert KT % kh_split == 0
    KH = KT // kh_split  # k subtiles per half-load

    mmdt = mybir.dt.float32r if mm_f32r else F32

    nc = bacc.Bacc("TRN2", target_bir_lowering=False, debug=bool(
        int(__import__("os").environ.get("BASS_DEBUG_BUILD", "0"))
    ))
    xT = nc.dram_tensor("xT", [K, B], mmdt, kind="ExternalInput").ap()
    wT = nc.dram_tensor("wT", [K, O], mmdt, kind="ExternalInput").ap()
    out = nc.dram_tensor("out", [B, O], F32, kind="ExternalOutput").ap()

    with tile.TileContext(nc) as tc:
        with (
            tc.tile_pool(name="wpool", bufs=1) as wpool,
            tc.tile_pool(name="misc", bufs=1) as misc,
            tc.tile_pool(name="wsqp", bufs=2) as wsqp,
            tc.tile_pool(name="xpool", bufs=6) as xpool,
            tc.tile_pool(name="sqpool", bufs=2) as sqpool,
            tc.tile_pool(name="accp", bufs=2) as accp,
            tc.tile_pool(name="normp", bufs=2) as normp,
            tc.tile_pool(name="opool", bufs=3) as opool,
            tc.tile_pool(name="psum", bufs=2, space="PSUM") as psump,
            tc.tile_pool(name="psum_n", bufs=2, space="PSUM") as psump_n,
        ):
            ones = misc.tile([P, 1], F32)
            nc.vector.memset(ones[:], 1.0)

            def load_xt(m):
                """Emit DMA loads of m-tile m's lhsT quarters (Sync ring)."""
                msl = slice(m * P, (m + 1) * P)
                xts = []
                for kh in range(kh_split):
                    xt = xpool.tile([P, KH, P], mmdt, tag="xt", name=f"xt{m}_{kh}")
                    nc.sync.dma_start(
                        xt[:],
                        xT[kh * KH * P : (kh + 1) * KH * P, msl].rearrange(
                            "(kt p) b -> p kt b", p=P
                        ),
                    )
                    xts.append(xt)
                return xts

            # prefetch the first m-tiles ahead of the bulk weight load so the
            # PE can start as soon as the first wT chunks land
            xt_prefetch = {m: load_xt(m) for m in range(min(2, MT))}

            # ---- resident weight shard, loaded per k-subtile (ACT ring, so
            # the 16MB bulk load doesn't sit ahead of the x stream) ----
            wT_sb = wpool.tile([P, KT, O], mmdt)
            for kt in range(KT):
                nc.scalar.dma_start(
                    wT_sb[:, kt, :],
                    wT[kt * P : (kt + 1) * P, :],
                )

            # ---- w norms: acc_w[p, o] = sum_kt wT[kt*P+p, o]^2 ----
            acc_w = misc.tile([P, O], F32)
            for kt in range(KT):
                wsq = wsqp.tile([P, O], F32, tag="wsq")
                nc.scalar.activation(wsq[:], wT_sb[:, kt, :].bitcast(F32), AF.Square)
                if kt == 0:
                    nc.vector.tensor_copy(acc_w[:], wsq[:])
                else:
                    nc.vector.tensor_add(acc_w[:], acc_w[:], wsq[:])

            # partition-reduce via ones-matmul, per 512-chunk (psum bank limit)
            invw_row = misc.tile([1, O], F32)
            for oc in range(OC):
                ps_w = psump_n.tile([1, NCHUNK], F32, tag="ps_w", name="ps_w")
                nc.tensor.matmul(
                    ps_w[:],
                    lhsT=ones[:],
                    rhs=acc_w[:, oc * NCHUNK : (oc + 1) * NCHUNK],
                    start=True,
                    stop=True,
                )
                wn = misc.tile([1, NCHUNK], F32, tag="wn")
                nc.scalar.activation(wn[:], ps_w[:], AF.Sqrt)
                nc.vector.tensor_scalar_max(wn[:], wn[:], EPS)
                nc.vector.reciprocal(
                    invw_row[:, oc * NCHUNK : (oc + 1) * NCHUNK], wn[:]
                )
            # broadcast to all partitions for the eviction multiply
            invw_b = misc.tile([P, O], F32)
            nc.gpsimd.partition_broadcast(invw_b[:], invw_row[:])

            # ---- main loop over output m-tiles ----
            for m in range(MT):
                msl = slice(m * P, (m + 1) * P)
                xts = xt_prefetch.pop(m, None) or load_xt(m)

                psums = [
                    psump.tile([P, NCHUNK], F32, tag=f"ps{oc}", name=f"ps{oc}")
                    for oc in range(OC)
                ]

                acchs = []
                for kh in range(kh_split):
                    xt = xts[kh]
                    for kt in range(KH):
                        for oc in range(OC):
                            nc.tensor.matmul(
                                psums[oc][:],
                                lhsT=xt[:, kt, :],
                                rhs=wT_sb[
                                    :,
                                    kh * KH + kt,
                                    oc * NCHUNK : (oc + 1) * NCHUNK,
                                ],
                                start=(kh == 0 and kt == 0),
                                stop=(kh == kh_split - 1 and kt == KH - 1),
                            )
                    # x-norm partials for this half
                    sq = sqpool.tile([P, KH, P], F32, tag="sq")
                    nc.scalar.activation(sq[:], xt[:].bitcast(F32), AF.Square)
                    acch = accp.tile([P, P], F32, tag=f"acch{kh}")
                    nc.vector.tensor_reduce(
                        acch[:], sq.rearrange("p kt b -> p b kt"), AX.X, ALU.add
                    )
                    acchs.append(acch)

                acc = acchs[0]
                for kh in range(1, kh_split):
                    acc2 = accp.tile([P, P], F32, tag="accsum")
                    nc.vector.tensor_add(acc2[:], acc[:], acchs[kh][:])
                    acc = acc2

                # sum over the 128 partitions: psum_n[b, 0] = ||x_b||^2
                ps_n = psump_n.tile([P, 1], F32, tag="ps_n", name="ps_n")
                nc.tensor.matmul(ps_n[:], lhsT=acc[:], rhs=ones[:], start=True, stop=True)
                xn = normp.tile([P, 1], F32, tag="xn")
                nc.scalar.activation(xn[:], ps_n[:], AF.Sqrt)
                nc.vector.tensor_scalar_max(xn[:], xn[:], EPS)
                invx = normp.tile([P, 1], F32, tag="invx")
                nc.vector.reciprocal(invx[:], xn[:])

                for oc in range(OC):
                    ot = opool.tile([P, NCHUNK], F32, tag="ot")
                    nc.vector.scalar_tensor_tensor(
                        out=ot[:],
                        in0=psums[oc][:],
                        scalar=invx[:],
                        in1=invw_b[:, oc * NCHUNK : (oc + 1) * NCHUNK],
                        op0=ALU.mult,
                        op1=ALU.mult,
                    )
                    nc.sync.dma_start(out[msl, oc * NCHUNK : (oc + 1) * NCHUNK], ot[:])

    nc.compile()
    return nc


_NC_CACHE = {}


def _get_nc(B, K, O, mm_f32r=True):
    key = (B, K, O, mm_f32r)
    if key not in _NC_CACHE:
        _NC_CACHE[key] = build_cosine_bass(B, K, O, mm_f32r=mm_f32r)
    return _NC_CACHE[key]


def _run(x, weights, mm_f32r=True, n_cores=N_CORES, trace=False, **kw):
    B, K = x.shape
    Ofull = weights.shape[0]
    assert weights.shape[1] == K
    Oshard = Ofull // n_cores

    xT = np.ascontiguousarray(x.T).astype(np.float32, copy=False)
    wT = np.ascontiguousarray(weights.T).astype(np.float32, copy=False)

    nc = _get_nc(B, K, Oshard, mm_f32r=mm_f32r)
    in_maps = [
        {
            "xT": xT,
            "wT": np.ascontiguousarray(wT[:, c * Oshard : (c + 1) * Oshard]),
        }
        for c in range(n_cores)
    ]
    res = run_bass_kernel_spmd(nc, in_maps, list(range(n_cores)), trace=trace, **kw)
    out = np.concatenate([res.results[c]["out"] for c in range(n_cores)], axis=1)
    return out, res


def kernel(x, weights):
    out, _ = _run(np.asarray(x), np.asarray(weights))
    return out
